# revision 12
# baseline (speedup 1.0000x reference)
"""MiniBatchDiscrimination Trainium2 kernel (8-core SPMD, circulant strips).

Reference computation:
    m = (x @ T).reshape(B, OUT_F, NUM_K)            # B=256, OUT_F=128, NUM_K=16
    dists = |m[None,:,:,:] - m[:,None,:,:]|         # [B, B, OUT_F, NUM_K]
    out = sum_i exp(-sum_k dists) - 1               # [B, OUT_F]
    return concat([x, out], axis=-1)                # [B, 640]

Strategy (identical SPMD program; per-core data = a column permutation):
  * The BxB pair matrix is covered once per unordered pair by 16 "strips":
    strip a = {i in 16-block a} x {j in 16-blocks a..a+8 (mod 16)}.  Each
    strip contributes row-sums for its j's (partial_b, sum over i) and,
    for the inner window blocks a+1..a+7 only, row-sums for its i's
    (partial_a, sum over j).  Exact cover: source-block offset e=(bj-bi)%16
    is counted by partial_b iff e in {0} u [8,15] and by partial_a iff
    e in [1,7].  Host accumulates partials from all cores and subtracts 1.
  * Core c owns strips 2c and 2c+1.  Their window union is 10 consecutive
    16-blocks -> 160 "virtual" columns; the host permutes x's rows per core
    so the program is core-independent (SPMD with full input replication
    of T; x columns gathered per core).
  * Per core work: GEMM m2[p=(f8,k), vcol, fo] (bf16, fo innermost so the
    pairwise subs run in DVE 2x mode), then per (strip, j-chunk of 32):
    tensor_sub (DVE 2x) -> |.| (split ACT Abs / DVE 4x sign-strip / GPSIMD
    sign-strip) -> k-sum on TensorE (block-diagonal ones, FD=512) -> Exp on
    ACT -> partial_b ones-matmul into packed PSUM stripes; partial_a via a
    small add-tree (GPSIMD) into SBUF slots.
"""

import os
import numpy as np

import concourse.bass as bass
import concourse.tile as tile
from concourse import bacc, mybir

BF16 = mybir.dt.bfloat16
FP32 = mybir.dt.float32
U16 = mybir.dt.uint16
NPBF16 = np.dtype(mybir.dt.np(BF16))

B = 256
IN_F = 512
OUT_F = 128
NUM_K = 16
N_CORES = 8
F8 = 8
FO = OUT_F // F8           # 16 fo groups (free dim)
KC = IN_F // 128           # 4 contraction chunks
NVB = 10                   # virtual 16-blocks per core
VCOLS = NVB * 16           # 160
NST = 2                    # strips per core
CH = [(0, 32), (32, 32), (64, 32), (96, 32), (128, 16)]  # (window j-off, len)

# abs engine per unit (st-major, then chunk, then i-half): A=ACT, G=GPSIMD,
# D=DVE.  Units 8,9,18,19 are the 16-j tail chunks.
_DEF = "AADAADADDA" "AADAADADAA"
ABS_SCHED = os.environ.get("ABS_SCHED", _DEF)
# engine for the partial_a add-tree: G (gpsimd) or D (DVE)
TREE_ENG = os.environ.get("TREE_ENG", "D")


def build_nc():
    nc = bacc.Bacc(name="mbd_strips")

    xT_d = nc.dram_tensor("xT", [128, KC, VCOLS], BF16, kind="ExternalInput")
    T_d = nc.dram_tensor("T_w", [FO, 128, KC, 128], BF16, kind="ExternalInput")
    onk_d = nc.dram_tensor("ones_k", [128, 8 * 64], BF16, kind="ExternalInput")
    ona_d = nc.dram_tensor("ones_acc", [128, F8], BF16, kind="ExternalInput")
    accB_d = nc.dram_tensor("accB", [128, 4, 512], FP32, kind="ExternalOutput")
    accA_d = nc.dram_tensor("accA", [128, NST, 4, FO], FP32, kind="ExternalOutput")

    with tile.TileContext(nc) as tc:
        with (
            tc.tile_pool(name="const", bufs=1) as constp,
            tc.tile_pool(name="gpsum", bufs=2, space=bass.MemorySpace.PSUM) as gps,
            tc.tile_pool(name="dpsum", bufs=2, space=bass.MemorySpace.PSUM) as dps,
            tc.tile_pool(name="bpsum", bufs=1, space=bass.MemorySpace.PSUM) as bps,
            tc.tile_pool(name="diffp", bufs=3) as dfp,
            tc.tile_pool(name="adp", bufs=3) as adp,
            tc.tile_pool(name="expp", bufs=3) as ep,
            tc.tile_pool(name="treep", bufs=2) as tp,
            tc.tile_pool(name="outp", bufs=1) as op_,
        ):
            zero_b = constp.tile([128, 1], FP32)
            nc.gpsimd.memset(zero_b[:], 0.0)

            ones_k = constp.tile([128, 8, 64], BF16)
            nc.sync.dma_start(ones_k[:], onk_d.rearrange("p (s q) -> p s q", q=64))
            ones_a = constp.tile([128, F8], BF16)
            nc.sync.dma_start(ones_a[:], ona_d[:])

            # warm the ACT exp/abs tables while DMAs land
            warm = constp.tile([128, 1], FP32)
            nc.scalar.activation(
                warm[:], zero_b[:], mybir.ActivationFunctionType.Exp, bias=zero_b[:]
            )

            xT_sb = constp.tile([128, KC, VCOLS], BF16)
            nc.sync.dma_start(xT_sb[:], xT_d[:])
            T_tiles = []
            dma_engs = [nc.sync, nc.scalar, nc.gpsimd]
            for fo in range(FO):
                tt = constp.tile([128, KC, 128], BF16, tag=f"T{fo}")
                dma_engs[fo % 3].dma_start(tt[:], T_d[fo])
                T_tiles.append(tt)

            # ---- GEMM: m2T[p, fo, vcol] -> rearrange -> m2[p, vcol, fo] ----
            m2T = constp.tile([128, FO, VCOLS], BF16)
            m2 = constp.tile([128, VCOLS, FO], BF16)
            for fo in range(FO):
                pm = gps.tile([128, VCOLS], FP32, tag="gemm")
                for c in range(KC):
                    nc.tensor.matmul(
                        pm[:],
                        T_tiles[fo][:, c, :],
                        xT_sb[:, c, :],
                        start=(c == 0),
                        stop=(c == KC - 1),
                    )
                nc.vector.tensor_copy(m2T[:, fo, :], pm[:])
            nc.scalar.copy(m2[:], m2T[:].rearrange("p f v -> p v f"))

            # ---- persistent accumulators ----
            accB = []
            for b in range(4):
                accB_t = bps.tile([128, 512], FP32, tag=f"accB{b}", name=f"accB{b}")
                accB.append(accB_t)
            accA_sb = op_.tile([128, NST, 4, FO], FP32)

            u = 0
            for st in range(NST):
                ivc = st * 16
                for ci, (joff, jn) in enumerate(CH):
                    jvc = st * 16 + joff
                    fd = jn * FO
                    pd = dps.tile([128, 512], FP32, tag="dist")
                    for h in range(2):
                        i0 = ivc + h * 8
                        diff = dfp.tile([128, 8, 32, FO], BF16, tag="diff")
                        nc.vector.tensor_sub(
                            diff[:, :, :jn, :],
                            m2[:, i0:i0 + 8, None, :].broadcast_to(
                                [128, 8, jn, FO]
                            ),
                            m2[:, None, jvc:jvc + jn, :].broadcast_to(
                                [128, 8, jn, FO]
                            ),
                        )
                        ad = adp.tile([128, 8, 32, FO], BF16, tag="absd")
                        eng = ABS_SCHED[u]
                        if eng == "A":
                            nc.scalar.activation(
                                ad[:, :, :jn, :], diff[:, :, :jn, :],
                                mybir.ActivationFunctionType.Abs, bias=zero_b[:],
                            )
                        elif eng == "G":
                            nc.gpsimd.tensor_scalar(
                                ad[:, :, :jn, :], diff[:, :, :jn, :],
                                0.0, None, op0=mybir.AluOpType.abs_max,
                            )
                        else:
                            nc.vector.tensor_scalar(
                                ad[:, :, :jn, :].bitcast(U16),
                                diff[:, :, :jn, :].bitcast(U16),
                                0x7FFF, None, op0=mybir.AluOpType.bitwise_and,
                            )
                        u += 1
                        for s8 in range(8):
                            nc.tensor.matmul(
                                pd[h * 64:(h + 1) * 64, :fd],
                                ones_k[:, s8, :],
                                ad[:, s8, :jn, :],
                                start=(s8 == 0),
                                stop=(s8 == 7),
                            )
                    et = ep.tile([128, 512], BF16, tag="expt")
                    nc.scalar.activation(
                        et[:, :fd], pd[:, :fd],
                        mybir.ActivationFunctionType.Exp, bias=zero_b[:], scale=-1.0,
                    )
                    # partial_b -> packed psum stripe
                    sidx = st * 5 + ci
                    poff = 32 * (sidx % 3)
                    nc.tensor.matmul(
                        accB[sidx // 3][poff:poff + 8, :fd],
                        ones_a[:],
                        et[:, :fd],
                        start=True,
                        stop=True,
                        skip_group_check=True,
                    )
                    # partial_a add-tree over eligible j (window blocks st+1..st+7)
                    lo = max(16 - joff, 0)
                    hi = min(128 - joff, jn)
                    if lo < hi:
                        # flat 2D views: j-slice [lo,hi) = cols [lo*FO, hi*FO)
                        cur = et[:, lo * FO: hi * FO]
                        n = hi - lo
                        lvl = 0
                        tadd = (nc.gpsimd.tensor_add if TREE_ENG == "G"
                                else nc.vector.tensor_add)
                        while n > 1:
                            half = n // 2
                            dt = BF16 if lvl < 2 else FP32
                            if half == 1:
                                nxt = accA_sb[:, st, ci, :]
                            else:
                                ntile = tp.tile([128, half * FO], dt,
                                                tag=f"tr{lvl}",
                                                name=f"tr{st}_{ci}_{lvl}")
                                nxt = ntile[:]
                            tadd(nxt, cur[:, :half * FO],
                                 cur[:, half * FO: 2 * half * FO])
                            cur = nxt
                            n = half
                            lvl += 1

            # ---- tail: drain accB psum, store ----
            fin = op_.tile([128, 4, 512], FP32)
            for b in range(4):
                nc.scalar.copy(fin[:, b, :], accB[b][:])
            nc.sync.dma_start(accB_d[:], fin[:])
            nc.sync.dma_start(accA_d[:], accA_sb[:])

    nc.finalize()
    return nc


def _vcol_real(c):
    """virtual column -> real row index, for core c."""
    vb = np.arange(VCOLS) // 16
    s = np.arange(VCOLS) % 16
    return ((2 * c + vb) % 16) * 16 + s


def make_in_maps(x: np.ndarray, T: np.ndarray):
    # xT_h[p, ch, i] = x[i, ch*128+p]
    xT_h = np.ascontiguousarray(
        x.T.astype(NPBF16).reshape(KC, 128, B).transpose(1, 0, 2)
    )
    T_b = np.ascontiguousarray(T).astype(NPBF16)  # [512, 2048]

    p = np.arange(128)[:, None]
    r = np.arange(F8)[None, :]
    ones_a = np.ascontiguousarray((p % 8 == r).astype(NPBF16))  # [128,8]
    q = np.arange(64)[None, None, :]
    s = np.arange(8)[None, :, None]
    ones_k = (q == s * 8 + p[:, :, None] // 16).astype(NPBF16)
    ones_k = np.ascontiguousarray(ones_k.reshape(128, 8 * 64))

    # T_w[fo, p, c, n] = T[c*128+p, fo*128+n]
    T_perm = np.ascontiguousarray(
        T_b.reshape(KC, 128, FO, 128).transpose(2, 1, 0, 3)
    )

    in_maps = []
    for c in range(N_CORES):
        cols = _vcol_real(c)
        in_maps.append({
            "xT": np.ascontiguousarray(xT_h[:, :, cols]),
            "T_w": T_perm,
            "ones_k": ones_k,
            "ones_acc": ones_a,
        })
    return in_maps


def assemble(x: np.ndarray, results) -> np.ndarray:
    out_pair = np.zeros((B, OUT_F), np.float32)
    for c, res in enumerate(results):
        cols = _vcol_real(c)
        accB = res["accB"].astype(np.float32)        # [128, 4, 512]
        accA = res["accA"].astype(np.float32)        # [128, NST, 4, FO]
        for st in range(NST):
            for ci, (joff, jn) in enumerate(CH):
                sidx = st * 5 + ci
                poff = 32 * (sidx % 3)
                vals = accB[poff:poff + 8, sidx // 3, :jn * FO].reshape(8, jn, FO)
                # out[j, fo*8+f8] += vals[f8, jj, fo]
                rows = cols[st * 16 + joff: st * 16 + joff + jn]
                out_pair[rows] += vals.transpose(1, 2, 0).reshape(jn, OUT_F)
            ta = accA[:, st, :, :].sum(axis=1)       # [128, FO]
            # row p = s*8+f8 -> out[i(s), fo*8+f8]
            rows = cols[st * 16: st * 16 + 16]
            out_pair[rows] += ta.reshape(16, 8, FO).transpose(0, 2, 1).reshape(
                16, OUT_F)
    out_pair -= 1.0
    out = np.empty((B, IN_F + OUT_F), np.float32)
    out[:, :IN_F] = x
    out[:, IN_F:] = out_pair
    return out


_NC_CACHE = None


def kernel(x: np.ndarray, T: np.ndarray) -> np.ndarray:
    global _NC_CACHE
    from concourse import bass_utils

    if _NC_CACHE is None:
        _NC_CACHE = build_nc()
    nc = _NC_CACHE
    in_maps = make_in_maps(np.asarray(x, np.float32), np.asarray(T, np.float32))
    res = bass_utils.run_bass_kernel_spmd(nc, in_maps, core_ids=list(range(N_CORES)))
    return assemble(np.asarray(x, np.float32), res.results)
